# revision 58
# baseline (speedup 1.0000x reference)
"""Trainium2 Bass kernel for EnhancedGraphSAGE (embed -> 2x SAGE-mean -> GAT -> MLP).

Self-contained: takes full inputs, shards node-wise across 8 NeuronCores
internally, returns the full [N, C] output.

v4 design (vs v2 baseline, 881us):
- One slot layout shared by ALL THREE aggregation phases: per dst block,
  16 edge-slot tiles grouped by src parity (gid%4 -> tiles 4q..4q+3). The
  one-hot masks (and their transposes) are loaded once and shared; most
  blocks' masks stay SBUF-resident.
- Tables are parity-packed so gather rows stay 256B-stride (the SWDGE
  minimum) without padding waste:
    L1: raw x, bf16, 2 nodes per 512B row (512B descriptors cost the same
        22.76ns as 256B ones), idx = gid//2.
    L2: h2 fp8, 4 nodes per 256B row, idx = gid//4. The packed table IS
        the AllGather output -> single AG, no repack, no int16 half-split.
    L3: (h3||el) fp8, 2 nodes per 256B row (68B valid each), idx =
        tidg//2; chunked AG + cheap strided HWDGE repack (68B rows).
- Collectives: 4 total (1 single-shot for L2's table, 3 chunks for GAT's)
  vs 10; cost model charges 15us fixed per collective.
- GAT softmax weighting runs DVE-heavy; Pool keeps only gather desc-gen.
"""

import numpy as np

import concourse.bacc as bacc
import concourse.bass as bass
import concourse.mybir as mybir
import concourse.tile as tile
from concourse.bass_utils import run_bass_kernel_spmd
from concourse.masks import make_identity

# Problem constants (hardcoded per spec)
N, E, IN, H, HEADS, C = 50000, 800000, 128, 64, 4, 40
SLOPE = 0.2

# Sharding geometry
NCORES = 8
NBLK = 56              # dst blocks per core
PB = 128               # dst slots per block
TPB = 16               # slot tiles per block (4 per parity class)
SLOTB = TPB * 128      # edge slots per block (2048)
PCAP = 4 * 128         # per-parity slot cap (4 tiles)
OWN = NBLK * PB        # own nodes per core (7168)
NID = NCORES * OWN     # internal id space (57344)
HALFG = NID // 2       # 28672 (L1/L3 packed row count)
QUARG = NID // 4       # 14336 (L2 packed row count)
D = 128                # x row width (bf16)
CH = 512               # dense chunk (nodes per matmul)
NCH_OWN = OWN // CH    # 14

# AllGather chunking (shared by the L2 and GAT tables): block ranges
GCHUNKS = [(0, 18), (18, 34), (34, 46), (46, 56)]
AG_FIRE = [min(b1 - 1 + 4, NBLK - 1) for (b0, b1) in GCHUNKS]
CP_AT = [-1 for f in AG_FIRE]

NRES = 33              # SBUF-resident mask blocks

F32 = mybir.dt.float32
BF16 = mybir.dt.bfloat16
FP8 = mybir.dt.float8e4
I16 = mybir.dt.int16
NP_BF16 = mybir.dt.np(BF16)
NP_FP8 = mybir.dt.np(FP8)

_cached = {}
DBG = False


def _build_bass(upto=99):
    nres = 20 if DBG else NRES
    nc = bacc.Bacc("TRN2", target_bir_lowering=False, debug=False,
                   num_devices=NCORES)

    # ---- I/O ----
    xq2 = nc.dram_tensor("xq2", [HALFG, 2 * D], BF16, kind="ExternalInput")
    xob = nc.dram_tensor("xob", [IN, OWN], BF16, kind="ExternalInput")
    idx1_in = nc.dram_tensor("idx1_in", [NBLK, 128, 2, 64], I16, kind="ExternalInput")
    idx2_in = nc.dram_tensor("idx2_in", [NBLK, 128, 2, 64], I16, kind="ExternalInput")
    idx3_in = nc.dram_tensor("idx3_in", [NBLK, 128, 2, 64], I16, kind="ExternalInput")
    mask_in = nc.dram_tensor("mask_in", [NBLK, 128, SLOTB], FP8, kind="ExternalInput")
    maskT_in = nc.dram_tensor("maskT_in", [NBLK, 128, SLOTB], FP8, kind="ExternalInput")
    dgib_in = nc.dram_tensor("dgib_in", [1, OWN], BF16, kind="ExternalInput")
    m1_in = nc.dram_tensor("m1_in", [1, OWN], BF16, kind="ExternalInput")

    wemb_in = nc.dram_tensor("wemb_in", [IN, H], BF16, kind="ExternalInput")
    bembc = nc.dram_tensor("bembc", [H, 1], F32, kind="ExternalInput")
    ws1_in = nc.dram_tensor("ws1_in", [H + 1, H], BF16, kind="ExternalInput")
    wq1_in = nc.dram_tensor("wq1_in", [IN, H], BF16, kind="ExternalInput")
    bn1_in = nc.dram_tensor("bn1_in", [H, 1], F32, kind="ExternalInput")
    ws2_in = nc.dram_tensor("ws2_in", [H, H], BF16, kind="ExternalInput")
    wn2_in = nc.dram_tensor("wn2_in", [H, H], BF16, kind="ExternalInput")
    bn2_in = nc.dram_tensor("bn2_in", [H, 1], F32, kind="ExternalInput")
    wl_in = nc.dram_tensor("wl_in", [H, HEADS], BF16, kind="ExternalInput")
    wr_in = nc.dram_tensor("wr_in", [H, HEADS], BF16, kind="ExternalInput")
    ulo_in = nc.dram_tensor("ulo_in", [128, H], BF16, kind="ExternalInput")
    uhi_in = nc.dram_tensor("uhi_in", [128, H], BF16, kind="ExternalInput")
    b1p = nc.dram_tensor("b1p", [H, 1], F32, kind="ExternalInput")
    w2_in = nc.dram_tensor("w2_in", [H, C], F32, kind="ExternalInput")
    b2c = nc.dram_tensor("b2c", [C, 1], F32, kind="ExternalInput")

    out = nc.dram_tensor("out", [OWN, C], F32, kind="ExternalOutput")
    if DBG:
        dbg_h2 = nc.dram_tensor("dbg_h2", [H, OWN], F32, kind="ExternalOutput")
        dbg_h3 = nc.dram_tensor("dbg_h3", [H, OWN], F32, kind="ExternalOutput")
        dbg_nb1 = nc.dram_tensor("dbg_nb1", [128, OWN], F32, kind="ExternalOutput")

    with tile.TileContext(nc) as tc:
        with (
            tc.tile_pool(name="wpool", bufs=1) as wp,
            tc.tile_pool(name="sbuf", bufs=3) as sb,
            tc.tile_pool(name="big", bufs=1) as bigp,
            tc.tile_pool(name="psum", bufs=2, space="PSUM") as pp,
            tc.tile_pool(name="dram", bufs=1, space="DRAM") as dram,
        ):
            # ---- weights resident in SBUF ----
            w_emb = wp.tile([IN, H], BF16); nc.scalar.dma_start(w_emb[:], wemb_in[:])
            b_embc = wp.tile([H, 1], F32); nc.scalar.dma_start(b_embc[:], bembc[:])
            w_s1 = wp.tile([H + 1, H], BF16); nc.scalar.dma_start(w_s1[:], ws1_in[:])
            w_q1 = wp.tile([IN, H], BF16); nc.scalar.dma_start(w_q1[:], wq1_in[:])
            b_n1 = wp.tile([H, 1], F32); nc.scalar.dma_start(b_n1[:], bn1_in[:])
            w_s2 = wp.tile([H, H], BF16); nc.scalar.dma_start(w_s2[:], ws2_in[:])
            w_n2 = wp.tile([H, H], BF16); nc.scalar.dma_start(w_n2[:], wn2_in[:])
            b_n2 = wp.tile([H, 1], F32); nc.scalar.dma_start(b_n2[:], bn2_in[:])
            w_lr = wp.tile([H, 2 * HEADS], BF16)
            nc.scalar.dma_start(w_lr[:, 0:HEADS], wl_in[:])
            nc.scalar.dma_start(w_lr[:, HEADS:], wr_in[:])
            u_lo = wp.tile([128, H], BF16); nc.scalar.dma_start(u_lo[:], ulo_in[:])
            u_hi = wp.tile([128, H], BF16); nc.scalar.dma_start(u_hi[:], uhi_in[:])
            b_1p = wp.tile([H, 1], F32); nc.scalar.dma_start(b_1p[:], b1p[:])
            w_2 = wp.tile([H, C], F32); nc.scalar.dma_start(w_2[:], w2_in[:])
            b_2 = wp.tile([C, 1], F32); nc.scalar.dma_start(b_2[:], b2c[:])

            id64b = wp.tile([64, 64], BF16)
            make_identity(nc, id64b[:])
            id128f = wp.tile([128, 128], F32)
            make_identity(nc, id128f[:])
            id128b = wp.tile([128, 128], BF16)
            nc.vector.tensor_copy(id128b[:], id128f[:])
            id40f = wp.tile([40, 40], F32)
            make_identity(nc, id40f[:])

            # deginv replicated across partitions: dgi_rep[p, n] = 1/deg(n)
            dgi_row = wp.tile([1, OWN], BF16)
            nc.scalar.dma_start(dgi_row[:], dgib_in[:])
            ones1 = wp.tile([1, 128], BF16)
            nc.vector.memset(ones1[:], 1.0)
            dgi_rep = bigp.tile([128, OWN], BF16)
            for ch in range(NCH_OWN):
                pdg = pp.tile([128, CH], F32, space="PSUM", tag="psB", bufs=3)
                nc.tensor.matmul(pdg[:], ones1[:], dgi_row[:, ch * CH:(ch + 1) * CH],
                                 start=True, stop=True)
                nc.scalar.activation(dgi_rep[:, ch * CH:(ch + 1) * CH], pdg[:],
                                     mybir.ActivationFunctionType.Identity)

            # resident masks: first NRES blocks' one-hot tiles stay in SBUF
            mask_res = bigp.tile([128, nres, SLOTB], FP8)

            # persistent feature planes (bf16, feat-major)
            h1T = bigp.tile([H + 1, OWN], BF16, tag="hT", bufs=2)
            nc.scalar.dma_start(h1T[H:H + 1, :], m1_in[:])
            h2T = bigp.tile([H, OWN], BF16, tag="hT", bufs=2)
            er_all = bigp.tile([128, NBLK, HEADS], BF16)

            # DRAM tables (tab2/tabg rows are chunk-major tid order)
            mine2 = [dram.tile([(b1 - b0) * 128, H], FP8, name=f"mine2_{ci}")
                     for ci, (b0, b1) in enumerate(GCHUNKS)]
            agc2 = [dram.tile([(b1 - b0) * 128 * NCORES, H], FP8,
                              addr_space="Shared", name=f"agc2_{ci}")
                    for ci, (b0, b1) in enumerate(GCHUNKS)]
            tab2 = dram.tile([NID, H], FP8, name="tab2")
            mineg = [dram.tile([(b1 - b0) * 128, H + HEADS], FP8,
                               name=f"mineg_{ci}")
                     for ci, (b0, b1) in enumerate(GCHUNKS)]
            agcg = [dram.tile([(b1 - b0) * 128 * NCORES, H + HEADS], FP8,
                              addr_space="Shared", name=f"agcg_{ci}")
                    for ci, (b0, b1) in enumerate(GCHUNKS)]
            # tabg rows are 256B: [a(68B) | pad(60) | b(68B) | pad(60)]
            tabg = dram.tile([HALFG, 256], FP8)

            def load_mask(b, tag="mk", bufs=4):
                if b < nres:
                    return mask_res[:, b, :]
                mkt = sb.tile([128, SLOTB], FP8, tag=tag, bufs=bufs)
                nc.sync.dma_start(mkt[:], mask_in[b])
                return mkt[:]

            def load_idx(b, idx_in, tag):
                it4 = sb.tile([128, 4, 2, 64], I16, tag=tag, bufs=2)
                nc.sync.dma_start(it4[:], idx_in[b:b + 4].rearrange(
                    "q p h s -> p q h s"))
                return it4

            # first L1 idx tiles + first resident masks before the bulk loads
            it_pre = {}
            it_pre[0] = load_idx(0, idx1_in, "it")
            nc.sync.dma_start(mask_res[:, 0:8, :],
                              mask_in[0:8].rearrange("b p s -> p b s"))
            it_pre[4] = load_idx(4, idx1_in, "it")

            # ============ own-feature dense embed: h1T = WembT xo + b ========
            QS = OWN // 4                          # 1792 = 3.5 * CH
            for hh in range(4):
                xb = sb.tile([IN, QS], BF16, tag="xb", bufs=2)
                nc.sync.dma_start(xb[:], xob[:, hh * QS:(hh + 1) * QS])
                for c0 in range(0, QS, CH):
                    cw = min(CH, QS - c0)
                    ph = pp.tile([H, CH], F32, space="PSUM", tag="psB", bufs=3)
                    nc.tensor.matmul(ph[:, 0:cw], w_emb[:], xb[:, c0:c0 + cw],
                                     start=True, stop=True)
                    co = hh * QS + c0
                    nc.scalar.activation(h1T[0:H, co:co + cw], ph[:, 0:cw],
                                         mybir.ActivationFunctionType.Identity,
                                         bias=b_embc[:], scale=1.0)
                if hh == 0:
                    nc.sync.dma_start(
                        mask_res[:, 8:20, :],
                        mask_in[8:20].rearrange("b p s -> p b s"))
                if hh == 1 and nres > 20:
                    nc.sync.dma_start(
                        mask_res[:, 20:nres, :],
                        mask_in[20:nres].rearrange("b p s -> p b s"))

            # ============ SAGE layer 1: aggregate raw x (2-pack 512B rows) ===
            it1 = None
            for b in range(NBLK if upto >= 1 else 0):
                if b % 4 == 0:
                    it1 = it_pre.pop(b, None) or load_idx(b, idx1_in, "it")
                mk = load_mask(b)
                g1 = sb.tile([128, TPB, 2 * D], BF16, tag="g", bufs=3)
                nc.gpsimd.dma_gather(g1[:, 0:TPB // 2, :], xq2[:],
                                     it1[:, b % 4, 0, :], 1024, 1024, 2 * D)
                nc.gpsimd.dma_gather(g1[:, TPB // 2:TPB, :], xq2[:],
                                     it1[:, b % 4, 1, :], 1024, 1024, 2 * D)
                pa = pp.tile([128, 128], F32, space="PSUM", tag="psAcc", bufs=2)
                for t in range(TPB):
                    off = ((t // 4) % 2) * D
                    nc.tensor.matmul(pa[:], g1[:, t, off:off + D],
                                     mk[:, t * 128:(t + 1) * 128],
                                     start=(t == 0), stop=(t == TPB - 1))
                bs = slice(b * 128, (b + 1) * 128)
                nb = sb.tile([128, 128], BF16, tag="nb")
                nc.vector.tensor_mul(nb[:], pa[:], dgi_rep[:, bs])
                if DBG:
                    nbf = sb.tile([128, 128], F32, tag="nbf")
                    nc.vector.tensor_copy(nbf[:], nb[:])
                    nc.sync.dma_start(dbg_nb1[:, bs], nbf[:])
                # dense: h2 = relu(Wq1^T nb + Ws1^T h1 + u1 m1 + bn1)
                p2 = pp.tile([H, 128], F32, space="PSUM", tag="psB", bufs=3)
                nc.tensor.matmul(p2[:], w_q1[:], nb[:], start=True, stop=False)
                nc.tensor.matmul(p2[:], w_s1[:], h1T[:, bs], start=False, stop=True)
                nc.scalar.activation(h2T[:, bs], p2[:],
                                     mybir.ActivationFunctionType.Relu,
                                     bias=b_n1[:], scale=1.0)
                # node-major fp8 rows for the single AG
                pw = pp.tile([128, H], BF16, space="PSUM", tag="psT", bufs=3)
                nc.tensor.transpose(pw[:], h2T[:, bs], id64b[:])
                stg = sb.tile([128, H], FP8, tag="stg2")
                nc.scalar.activation(stg[:], pw[:],
                                     mybir.ActivationFunctionType.Identity)
                ci_b = next(ci for ci, (b0, b1) in enumerate(GCHUNKS)
                            if b0 <= b < b1)
                mb0 = GCHUNKS[ci_b][0] * 128
                nc.scalar.dma_start(
                    mine2[ci_b][b * 128 - mb0:(b + 1) * 128 - mb0, :], stg[:])
                # chunked AllGather, emitted a few blocks late so the Pool
                # wait on the chunk's mine writes is already satisfied
                for ci, (b0, b1) in enumerate(GCHUNKS):
                    if b == AG_FIRE[ci]:
                        nc.gpsimd.collective_compute(
                            "AllGather", mybir.AluOpType.bypass,
                            replica_groups=[list(range(NCORES))],
                            ins=[mine2[ci][:]], outs=[agc2[ci][:]],
                        )
                    if b == CP_AT[ci]:
                        r0 = b0 * 128 * NCORES
                        r1 = b1 * 128 * NCORES
                        nc.gpsimd.dma_start(tab2[r0:r1, :], agc2[ci][:])
            # remaining chunk->table copies at loop end on Pool: Pool's next
            # work is L2's gathers, which need the full table anyway
            for ci, (b0, b1) in enumerate(GCHUNKS):
                if upto >= 1 and CP_AT[ci] < 0:
                    r0 = b0 * 128 * NCORES
                    r1 = b1 * 128 * NCORES
                    nc.gpsimd.dma_start(tab2[r0:r1, :], agc2[ci][:])
            tab2v = tab2[:].rearrange("(r q) w -> r (q w)", q=4)  # [QUARG, 256]

            # ============ SAGE layer 2: aggregate h2 (4-pack fp8 rows) =======
            h3T = bigp.tile([H, OWN], BF16, tag="hT", bufs=2)
            it2 = None
            for b in range(NBLK if upto >= 2 else 0):
                if b % 4 == 0:
                    it2 = load_idx(b, idx2_in, "it")
                mk = load_mask(b)
                g2 = sb.tile([128, TPB, 4 * H], FP8, tag="g", bufs=3)
                nc.gpsimd.dma_gather(g2[:, 0:TPB // 2, :], tab2v,
                                     it2[:, b % 4, 0, :], 1024, 1024, 4 * H)
                nc.gpsimd.dma_gather(g2[:, TPB // 2:TPB, :], tab2v,
                                     it2[:, b % 4, 1, :], 1024, 1024, 4 * H)
                pa = pp.tile([H, 128], F32, space="PSUM", tag="psAcc", bufs=2)
                for t in range(TPB):
                    off = (t // 4) * H
                    nc.tensor.matmul(pa[:], g2[:, t, off:off + H],
                                     mk[:, t * 128:(t + 1) * 128],
                                     start=(t == 0), stop=(t == TPB - 1))
                bs = slice(b * 128, (b + 1) * 128)
                nb = sb.tile([H, 128], BF16, tag="nb")
                nc.vector.tensor_mul(nb[:], pa[:], dgi_rep[0:H, bs])
                p2 = pp.tile([H, 128], F32, space="PSUM", tag="psB", bufs=3)
                nc.tensor.matmul(p2[:], w_n2[:], nb[:], start=True, stop=False)
                nc.tensor.matmul(p2[:], w_s2[:], h2T[:, bs], start=False, stop=True)
                nc.scalar.activation(h3T[:, bs], p2[:],
                                     mybir.ActivationFunctionType.Relu,
                                     bias=b_n2[:], scale=1.0)
                # stg rows: h3 (fp8) || el (fp8); er kept on-core
                pw = pp.tile([128, H], BF16, space="PSUM", tag="psT", bufs=3)
                nc.tensor.transpose(pw[:], h3T[:, bs], id64b[:])
                stg = sb.tile([128, H + HEADS], FP8, tag="stg2")
                nc.scalar.activation(stg[:, 0:H], pw[:],
                                     mybir.ActivationFunctionType.Identity)
                pelr = pp.tile([128, 2 * HEADS], F32, space="PSUM", tag="psT", bufs=3)
                nc.tensor.matmul(pelr[:], h3T[:, bs], w_lr[:], start=True, stop=True)
                nc.scalar.activation(stg[:, H:], pelr[:, 0:HEADS],
                                     mybir.ActivationFunctionType.Identity)
                nc.scalar.activation(er_all[:, b, :], pelr[:, HEADS:],
                                     mybir.ActivationFunctionType.Identity)
                ci_b = next(ci for ci, (b0, b1) in enumerate(GCHUNKS)
                            if b0 <= b < b1)
                mb0 = GCHUNKS[ci_b][0] * 128
                nc.scalar.dma_start(
                    mineg[ci_b][b * 128 - mb0:(b + 1) * 128 - mb0, :], stg[:])
                # chunked AllGather, emitted a few blocks late
                for ci, (b0, b1) in enumerate(GCHUNKS):
                    if b == AG_FIRE[ci]:
                        nc.gpsimd.collective_compute(
                            "AllGather", mybir.AluOpType.bypass,
                            replica_groups=[list(range(NCORES))],
                            ins=[mineg[ci][:]], outs=[agcg[ci][:]],
                        )
            # strided 2-pack repacks at loop end, halves split across the
            # Pool and SP queues (SWDGE desc-gen overlaps HWDGE transfers)
            for ci, (b0, b1) in enumerate(GCHUNKS):
                if upto >= 2:
                    g0 = b0 * 128 * NCORES
                    g1r = b1 * 128 * NCORES
                    gm = (g0 + g1r) // 2
                    rows = (g1r - g0) // 2
                    tv = tabg[:].rearrange("r (q w) -> (r q) w", q=2)
                    nc.gpsimd.dma_start(tv[g0:gm, 0:H + HEADS],
                                        agcg[ci][0:rows, :])
                    nc.gpsimd.dma_start(tv[gm:g1r, 0:H + HEADS],
                                        agcg[ci][rows:2 * rows, :])

            if DBG:
                for ch in range(NCH_OWN):
                    cs = slice(ch * CH, (ch + 1) * CH)
                    t2 = sb.tile([H, CH], F32, tag="dbg2")
                    nc.vector.tensor_copy(t2[:], h2T[:, cs])
                    nc.sync.dma_start(dbg_h2[:, cs], t2[:])
                    t3 = sb.tile([H, CH], F32, tag="dbg3")
                    nc.vector.tensor_copy(t3[:], h3T[:, cs])
                    nc.sync.dma_start(dbg_h3[:, cs], t3[:])

            # ============ GAT dense + classifier (per 4-block chunk) ========
            def og_stage(og4, half):
                stgT = sb.tile([128, CH], BF16, tag=f"ogs{half}", bufs=2)
                for q in range(4):
                    ptg = pp.tile([128, 128], BF16, space="PSUM", tag="psAcc", bufs=2)
                    nc.tensor.transpose(
                        ptg[:], og4[:, q, half * 128:(half + 1) * 128], id128b[:])
                    nc.scalar.activation(stgT[:, q * 128:(q + 1) * 128], ptg[:],
                                         mybir.ActivationFunctionType.Identity)
                return stgT

            def cls_chunk(ch, og4):
                og_loS = og_stage(og4, 0)
                og_hiS = og_stage(og4, 1)
                p4 = pp.tile([H, CH], F32, space="PSUM", tag="psAcc", bufs=2)
                nc.tensor.matmul(p4[:], u_lo[:], og_loS[:],
                                 start=True, stop=False)
                nc.tensor.matmul(p4[:], u_hi[:], og_hiS[:],
                                 start=False, stop=True)
                h4 = sb.tile([H, CH], F32, tag="h4", bufs=2)
                nc.scalar.activation(h4[:], p4[:],
                                     mybir.ActivationFunctionType.Relu,
                                     bias=b_1p[:], scale=1.0)
                plg = pp.tile([C, CH], F32, space="PSUM", tag="psAcc", bufs=2)
                nc.tensor.matmul(plg[:], w_2[:], h4[:], start=True, stop=True)
                lg = sb.tile([C, CH], F32, tag="lg", bufs=2)
                nc.scalar.activation(lg[:], plg[:],
                                     mybir.ActivationFunctionType.Identity,
                                     bias=b_2[:], scale=1.0)
                ostg = sb.tile([128, 4, C], F32, tag="ostg")
                for q in range(4):
                    plt = pp.tile([128, C], F32, space="PSUM", tag="psAcc", bufs=2)
                    nc.tensor.transpose(plt[:], lg[:, q * 128:(q + 1) * 128], id40f[:])
                    nc.vector.tensor_copy(ostg[:, q, :], plt[:])
                nc.scalar.dma_start(
                    out[ch * CH:(ch + 1) * CH, :].rearrange("(q p) c -> p q c", p=128),
                    ostg[:])

            # ================= GAT aggregation =================
            WST = HEADS * H + HEADS
            it3 = None
            og4 = None
            for b in range(NBLK if upto >= 3 else 0):
                if b % 4 == 0:
                    it3 = load_idx(b, idx3_in, "it")
                    og4 = sb.tile([128, 4, HEADS * H], BF16, tag="og4", bufs=2)
                mk = load_mask(b)
                mt = sb.tile([128, SLOTB], FP8, tag="mt", bufs=3)
                nc.sync.dma_start(mt[:], maskT_in[b])
                g3 = sb.tile([128, TPB, 256], FP8, tag="g", bufs=3)
                nc.gpsimd.dma_gather(g3[:, 0:TPB // 2, :], tabg[:],
                                     it3[:, b % 4, 0, :], 1024, 1024, 256)
                nc.gpsimd.dma_gather(g3[:, TPB // 2:TPB, :], tabg[:],
                                     it3[:, b % 4, 1, :], 1024, 1024, 256)
                # er broadcast to edge slots via maskT matmuls
                perb = pp.tile([128, TPB, HEADS], F32, space="PSUM", tag="psT", bufs=3)
                for t in range(TPB):
                    nc.tensor.matmul(perb[:, t, :], mt[:, t * 128:(t + 1) * 128],
                                     er_all[:, b, :], start=True, stop=True)
                # e = leaky_relu(el + er); per-parity el offset (64 or 192)
                ee = sb.tile([128, TPB, HEADS], BF16, tag="ee")
                g3p = g3[:].rearrange("p (a c) w -> p a c w", a=2)  # a: par pair
                eep = ee[:].rearrange("p (a c) h -> p a c h", a=2)
                for pc in range(2):  # parity class: tiles 0-3,8-11 / 4-7,12-15
                    eoff = (pc % 2) * 128 + H
                    nc.vector.tensor_add(
                        eep[:, :, pc * 4:(pc + 1) * 4, :],
                        g3p[:, :, pc * 4:(pc + 1) * 4, eoff:eoff + HEADS],
                        perb[:].rearrange("p (a c) h -> p a c h", a=2)[
                            :, :, pc * 4:(pc + 1) * 4, :])
                nc.vector.scalar_tensor_tensor(
                    ee[:], ee[:], SLOPE, ee[:],
                    mybir.AluOpType.mult, mybir.AluOpType.max)
                wst = sb.tile([128, TPB, WST], BF16, tag="wst", bufs=2)
                nc.scalar.activation(wst[:, :, HEADS * H:], ee[:],
                                     mybir.ActivationFunctionType.Exp)
                # fused per-head weighting: wst[p,t,h,f] = g[p,t,f]*ex[p,t,h]
                # 4 ops (parity-class x call-half); 3 on DVE, 1 on Pool
                wstp = wst[:].rearrange("p (a c) w -> p a c w", a=2)
                for pc in range(2):
                    hoff = (pc % 2) * 128
                    for a in range(2):
                        weng = (nc.gpsimd if (pc, a) == (1, 1) and b % 2 == 0
                                else nc.vector)
                        weng.tensor_mul(
                            wstp[:, a, pc * 4:(pc + 1) * 4, 0:HEADS * H].rearrange(
                                "p c (h f) -> p c h f", h=HEADS),
                            g3p[:, a, pc * 4:(pc + 1) * 4, hoff:hoff + H].rearrange(
                                "p c (o f) -> p c o f", o=1).to_broadcast(
                                    [128, 4, HEADS, H]),
                            wstp[:, a, pc * 4:(pc + 1) * 4, HEADS * H:].rearrange(
                                "p c (h o) -> p c h o", o=1).to_broadcast(
                                    [128, 4, HEADS, H]))
                pg = pp.tile([128, WST], F32, space="PSUM", tag="psB", bufs=3)
                for t in range(TPB):
                    nc.tensor.matmul(pg[:], mk[:, t * 128:(t + 1) * 128],
                                     wst[:, t, :], start=(t == 0),
                                     stop=(t == TPB - 1))
                # normalize by z
                zt = sb.tile([128, HEADS], F32, tag="zt")
                nc.vector.tensor_scalar_max(zt[:], pg[:, HEADS * H:], 1e-20)
                zi = sb.tile([128, HEADS], F32, tag="zi")
                nc.vector.reciprocal(zi[:], zt[:])
                for hh2 in range(HEADS):
                    nc.scalar.activation(
                        og4[:, b % 4, hh2 * H:(hh2 + 1) * H],
                        pg[:, hh2 * H:(hh2 + 1) * H],
                        mybir.ActivationFunctionType.Identity,
                        scale=zi[:, hh2:hh2 + 1])
                # classifier chunk interleaved as soon as its 4 blocks exist
                if b % 4 == 3 and upto >= 4:
                    cls_chunk(b // 4, og4)

            if upto < 4:
                zo = sb.tile([128, NBLK, C], F32, tag="zo")
                nc.vector.memset(zo[:], 0.0)
                nc.sync.dma_start(
                    out[:].rearrange("(q p) c -> p q c", p=128), zo[:])

    nc.compile()
    return nc


def _tidg_of(gid):
    """GAT-chunk-major table row id for a global-internal node id."""
    core = gid // OWN
    local = gid % OWN
    blk = local // 128
    p = local % 128
    tid = np.zeros_like(gid)
    for (b0, b1) in GCHUNKS:
        r0 = b0 * 128
        g0 = r0 * NCORES
        cr = (b1 - b0) * 128
        m = (blk >= b0) & (blk < b1)
        tid[m] = g0 + core[m] * cr + (blk[m] - b0) * 128 + p[m]
    return tid


def _plan(src, dst):
    """Host-side graph partitioning. Returns per-core index/mask arrays.

    Slot layout (shared by all 3 phases): per dst block, 16 tiles of 128
    slots; tiles 4q..4q+3 hold edges whose src parity (gid%4) == q.
    """
    src = np.asarray(src).astype(np.int64)
    dst = np.asarray(dst).astype(np.int64)
    perm = None
    for seed in range(8):
        rng = np.random.default_rng(seed)
        cand = rng.permutation(NID)[:N].astype(np.int64)  # orig -> gid
        inv = np.full(NID, -1, np.int64)                  # gid -> orig
        inv[cand] = np.arange(N)
        ok = False
        for _ in range(200):
            si = cand[src]
            di = cand[dst]
            gblk = di // PB                      # 0..447 global dst block
            par = si % 4
            grp = gblk * 4 + par
            cnt = np.bincount(grp, minlength=NCORES * NBLK * 4)
            over = np.nonzero(cnt > PCAP)[0]
            if len(over) == 0:
                ok = True
                break
            # repair: for the worst cell, swap a contributing src's gid with
            # a same-128-block gid of the cell's minority parity (dst
            # structure is untouched; only parity classes change).
            gcell = over[np.argmax(cnt[over])]
            bad_b, bad_q = gcell // 4, gcell % 4
            cands = np.nonzero((gblk == bad_b) & (par == bad_q))[0]
            u = si[cands[rng.integers(len(cands))]]
            qstar = int(np.argmin(cnt[bad_b * 4:(bad_b + 1) * 4]))
            ublk = (u // 128) * 128
            wp = np.nonzero((np.arange(128) + ublk) % 4 == qstar)[0]
            w = ublk + wp[rng.integers(len(wp))]
            ou, ow = inv[u], inv[w]
            if ou >= 0:
                cand[ou] = w
            if ow >= 0:
                cand[ow] = u
            inv[u], inv[w] = ow, ou
        if ok:
            perm = cand
            break
    if perm is None:
        raise RuntimeError("parity-cap repair did not converge")

    si = perm[src]
    di = perm[dst]
    gblk = di // PB
    par = si % 4
    grp = gblk * 4 + par
    cnt = np.bincount(grp, minlength=NCORES * NBLK * 4)
    assert cnt.max() <= PCAP

    order = np.lexsort((si, grp))
    g_sorted = grp[order]
    starts = np.zeros(NCORES * NBLK * 4 + 1, np.int64)
    np.cumsum(cnt, out=starts[1:])
    j_in_grp = np.arange(E, dtype=np.int64) - starts[g_sorted]

    e_si = si[order]
    e_di = di[order]
    e_par = par[order]
    e_gblk = gblk[order]
    e_core = e_gblk // NBLK
    e_blk = e_gblk % NBLK

    # slot position: tile t = 4*par + j//128, p = j%128
    t_of = e_par * 4 + j_in_grp // 128
    p_of = j_in_grp % 128
    # within-call linear slot index (call 0: tiles 0-7, call 1: tiles 8-15)
    call = (t_of >= 8).astype(np.int64)
    j2 = (t_of % 8) * 128 + p_of

    tidg = _tidg_of(e_si)

    def pack_idx(val):
        """[NCORES, NBLK, 128, 2, 64] int16, idx[j2%16, j2//16] per call."""
        a = np.zeros((NCORES, NBLK, 16, 2, 64), np.int16)
        a[e_core, e_blk, j2 % 16, call, j2 // 16] = val.astype(np.int16)
        a = np.broadcast_to(a[:, :, None], (NCORES, NBLK, 8, 16, 2, 64))
        return a.reshape(NCORES, NBLK, 128, 2, 64).copy()

    idx1 = pack_idx(e_si // 2)
    idx2 = pack_idx(tidg // 4)
    idx3 = pack_idx(tidg // 2)

    # masks [NCORES, NBLK, 128, TPB*128] fp8: slot (t, p) -> dst col d
    d_of = e_di % PB
    m8 = np.zeros((NCORES, NBLK, 128, SLOTB), np.uint8)
    one_fp8 = np.array(1.0, NP_FP8).view(np.uint8).item()
    m8[e_core, e_blk, p_of, t_of * 128 + d_of] = one_fp8
    mT8 = m8.reshape(NCORES, NBLK, 128, TPB, 128).transpose(0, 1, 4, 3, 2)
    mT8 = np.ascontiguousarray(mT8).reshape(NCORES, NBLK, 128, SLOTB)

    # deginv per dst slot
    deg = np.bincount(di, minlength=NID).astype(np.float32)
    dgi = (1.0 / np.maximum(deg, 1.0)).reshape(NCORES, 1, OWN)
    m1 = (deg > 0).astype(np.float32).reshape(NCORES, 1, OWN)

    return perm, idx1, idx2, idx3, m8.view(NP_FP8), mT8.view(NP_FP8), dgi, m1


def kernel(x, src, dst, W_embed, b_embed, Ws1, Wn1, bn1, Ws2, Wn2, bn2,
           Wg, al, ar, bg, W1, b1, W2, b2):
    x = np.asarray(x, np.float32)
    key = (src.shape[0], int(np.asarray(src)[:64].sum()),
           int(np.asarray(dst)[:64].sum()))
    if _cached.get("plan_key") != key:
        _cached["plan"] = _plan(src, dst)
        _cached["plan_key"] = key
    perm, idx1, idx2, idx3, m8, mT8, dgi, m1 = _cached["plan"]

    if "nc" not in _cached:
        _cached["nc"] = _build_bass()
    nc = _cached["nc"]

    # weight preprocessing
    Wemb = np.asarray(W_embed, np.float32)
    bemb = np.asarray(b_embed, np.float32)
    Wn1 = np.asarray(Wn1, np.float32)
    Wg = np.asarray(Wg, np.float32)
    al = np.asarray(al, np.float32)
    ar = np.asarray(ar, np.float32)
    W1 = np.asarray(W1, np.float32)
    WL = np.stack([Wg[:, h * H:(h + 1) * H] @ al[h] for h in range(HEADS)], 1)
    WR = np.stack([Wg[:, h * H:(h + 1) * H] @ ar[h] for h in range(HEADS)], 1)
    b1p = (np.asarray(b1, np.float32) + np.asarray(bg, np.float32) @ W1)
    U = [Wg[:, h * H:(h + 1) * H] @ W1[h * H:(h + 1) * H] for h in range(HEADS)]
    Ulo = np.vstack([U[0], U[1]]).astype(NP_BF16)
    Uhi = np.vstack([U[2], U[3]]).astype(NP_BF16)
    Wq1 = (Wemb @ Wn1).astype(NP_BF16)           # [IN, H]
    u1row = (bemb @ Wn1).reshape(1, H).astype(NP_BF16)

    # x table: 2-pack rows [HALFG, 256] bf16, gid order
    xfull = np.zeros((NID, D), NP_BF16)
    xfull[perm] = x.astype(NP_BF16)
    xq2 = xfull.reshape(HALFG, 2 * D)
    xT = np.zeros((IN, NID), np.float32)
    xT[:, perm] = x.T

    common = {
        "xq2": xq2,
        "wemb_in": Wemb.astype(NP_BF16),
        "bembc": bemb.reshape(H, 1),
        "ws1_in": np.vstack([np.asarray(Ws1, np.float32), u1row.astype(np.float32)]).astype(NP_BF16),
        "wq1_in": Wq1,
        "bn1_in": np.asarray(bn1, np.float32).reshape(H, 1),
        "ws2_in": np.asarray(Ws2, np.float32).astype(NP_BF16),
        "wn2_in": np.asarray(Wn2, np.float32).astype(NP_BF16),
        "bn2_in": np.asarray(bn2, np.float32).reshape(H, 1),
        "wl_in": WL.astype(NP_BF16), "wr_in": WR.astype(NP_BF16),
        "ulo_in": Ulo, "uhi_in": Uhi,
        "b1p": b1p.reshape(H, 1),
        "w2_in": np.asarray(W2, np.float32),
        "b2c": np.asarray(b2, np.float32).reshape(C, 1),
    }
    in_maps = []
    for c in range(NCORES):
        m = dict(common)
        m["xob"] = np.ascontiguousarray(
            xT[:, c * OWN:(c + 1) * OWN]).astype(NP_BF16)
        m["idx1_in"] = np.ascontiguousarray(idx1[c])
        m["idx2_in"] = np.ascontiguousarray(idx2[c])
        m["idx3_in"] = np.ascontiguousarray(idx3[c])
        m["mask_in"] = np.ascontiguousarray(m8[c])
        m["maskT_in"] = np.ascontiguousarray(mT8[c])
        m["dgib_in"] = np.ascontiguousarray(
            dgi[c].reshape(1, OWN)).astype(NP_BF16)
        m["m1_in"] = np.ascontiguousarray(m1[c]).astype(NP_BF16)
        in_maps.append(m)

    res = run_bass_kernel_spmd(nc, in_maps, core_ids=list(range(NCORES)))
    full = np.concatenate([res.results[c]["out"] for c in range(NCORES)], 0)
    return full[perm].astype(np.float32)
